# revision 5
# baseline (speedup 1.0000x reference)
"""HANConv Trainium2 kernel (8 NeuronCores, SPMD, full-I/O contract).

Strategy
--------
Destination-sharded, fully core-independent:
  * Windows (128 dst rows) of both relations are sorted by edge-block
    count and dealt round-robin to (slot, core) so every core's slot i
    has a near-identical block count; per-slot counts are exact (no
    global max padding).  The same program (per-slot counts baked in)
    runs SPMD on all 8 cores; per-core data (indices, one-hot columns,
    degree reciprocals, self-feature slices, output row mapping) is
    permuted on host.
  * Per slot, source rows are gathered from an fp8 copy of the raw
    source features via gpsimd.dma_gather (int16 indices => lo/hi
    table split) and segment-summed with one-hot matmuls (fp8 one-hot
    x fp8 gathered rows) accumulating in PSUM.
  * Aggregating RAW features (M = A @ x) lets the relation transform
    and the semantic-score transform become one 512-wide dense matmul
    from M^T with host-concatenated weights [W_rel | W_rel @ W_sem];
    the self path likewise uses [W_self | W_self @ W_sem].  Degree
    normalization commutes with the right-multiplication and is applied
    after the dense matmul.
  * 2-candidate semantic softmax == sigmoid of score difference.
  * All host-prepped operands are packed into 4 DRAM tensors (one per
    dtype) and the two outputs into one, minimizing per-dispatch
    argument-marshaling overhead.
"""

import sys

sys.path.insert(0, "/opt/trn_rl_repo")

import numpy as np
import ml_dtypes

import concourse.bacc as bacc
import concourse.mybir as mybir
import concourse.tile as tile
from concourse.bass_utils import run_bass_kernel_spmd

P = 128
N = 50000
D = 256
HALF = 32768  # int16 gather index limit
NCORES = 8
NW_TOTAL = (N + P - 1) // P            # 391 destination windows
NWIN = (NW_TOTAL + NCORES - 1) // NCORES  # 49 slots per core
NW_ALLOC = NWIN * NCORES               # 392 (incl. 1 phantom window)
NPAD = NWIN * P                        # 6272 output rows per core per rel

GDT = mybir.dt.float8e4               # gather table / one-hot dtype
GNP = mybir.dt.np(GDT)
BF16 = ml_dtypes.bfloat16
F32 = np.float32


# ---------------------------------------------------------------- host prep
def _prep_relation(row, col):
    """Sort edges by (dst window, src half); deal windows to (slot, core)
    balanced by block count; pack per-core gather indices / one-hot cols /
    reciprocals with exact per-slot block counts."""
    hi = (row >= HALF).astype(np.int64)
    w_of = (col // P).astype(np.int64)
    key = w_of * 2 + hi
    order = np.argsort(key, kind="stable")
    rs = row[order].astype(np.int64)
    cs = col[order].astype(np.int64)
    counts = np.bincount(key, minlength=NW_TOTAL * 2)
    grp_start = np.zeros(NW_TOTAL * 2 + 1, np.int64)
    np.cumsum(counts, out=grp_start[1:])
    lo_cnt = np.concatenate([counts[0::2], np.zeros(NW_ALLOC - NW_TOTAL, np.int64)])
    hi_cnt = np.concatenate([counts[1::2], np.zeros(NW_ALLOC - NW_TOTAL, np.int64)])
    bl = -(-lo_cnt // P)
    bh = -(-hi_cnt // P)

    order_w = np.argsort(-(bl + bh), kind="stable")
    wlist = order_w.reshape(NWIN, NCORES)      # [slot, core] -> window id
    BL = bl[wlist].max(axis=1)
    BH = bh[wlist].max(axis=1)
    BL[(BL + BH) == 0] = 1
    call = BL + BH
    C = int(call.sum())
    coff = np.zeros(NWIN, np.int64)
    np.cumsum(call[:-1], out=coff[1:])

    deg = np.bincount(col, minlength=NW_ALLOC * P).astype(F32)[: NW_ALLOC * P]
    rec_full = 1.0 / np.maximum(deg, 1.0)

    idxs, colfs, recips = [], [], []
    for c in range(NCORES):
        idxv = np.zeros(C * P, np.int16)
        colv = np.full(C * P, -1.0, np.float32)
        recip = np.empty((P, NWIN), np.float32)
        for i in range(NWIN):
            w = int(wlist[i, c])
            base = int(coff[i]) * P
            if w < NW_TOTAL:
                nlo = int(lo_cnt[w])
                l0 = int(grp_start[2 * w])
                idxv[base: base + nlo] = rs[l0: l0 + nlo]
                colv[base: base + nlo] = cs[l0: l0 + nlo] - w * P
                hbase = base + int(BL[i]) * P
                nhi = int(hi_cnt[w])
                h0 = int(grp_start[2 * w + 1])
                idxv[hbase: hbase + nhi] = rs[h0: h0 + nhi] - HALF
                colv[hbase: hbase + nhi] = cs[h0: h0 + nhi] - w * P
            recip[:, i] = rec_full[w * P: (w + 1) * P]
        idx16 = np.ascontiguousarray(np.tile(idxv.reshape(C * 8, 16).T, (8, 1)))
        colf = np.ascontiguousarray(colv.reshape(C, P).T).astype(BF16)
        idxs.append(idx16)
        colfs.append(colf)
        recips.append(recip)
    return dict(wlist=wlist, BL=tuple(int(x) for x in BL),
                BH=tuple(int(x) for x in BH), C=C, coff=coff,
                idx=idxs, colf=colfs, recip=recips)


def _layout16(C_wr, C_wn):
    L, off = {}, 0
    for name, w in [("iota", P), ("ident", P),
                    ("wp_wr0", 512), ("wp_wr1", 512),
                    ("wp_wn0", 512), ("wp_wn1", 512),
                    ("wp_sp0", 512), ("wp_sp1", 512),
                    ("wp_sa0", 512), ("wp_sa1", 512),
                    ("colf_wr", C_wr), ("colf_wn", C_wn),
                    ("xtp0", NPAD), ("xtp1", NPAD),
                    ("xta0", NPAD), ("xta1", NPAD)]:
        L[name] = (off, w)
        off += w
    return L, off


def _layoutf():
    L, off = {}, 0
    for name, w in [("bp_p", 512), ("bp_a", 512), ("bsem", D), ("wsc", D),
                    ("recip_wr", NWIN), ("recip_wn", NWIN)]:
        L[name] = (off, w)
        off += w
    return L, off


def _host_prep(inp):
    pr = {}
    pr["wr"] = _prep_relation(np.asarray(inp["row_writes"]), np.asarray(inp["col_writes"]))
    pr["wn"] = _prep_relation(np.asarray(inp["row_written"]), np.asarray(inp["col_written"]))

    xa = np.asarray(inp["x_author"], dtype=F32)
    xp = np.asarray(inp["x_paper"], dtype=F32)
    xb = np.empty((2 * N, D), dtype=GNP)
    xb[:N] = xa.astype(GNP)
    xb[N:] = xp.astype(GNP)
    pr["xb"] = xb

    W_sem = np.asarray(inp["W_sem"], dtype=F32)
    b_sem = np.asarray(inp["b_sem"], dtype=F32)
    w_score = np.asarray(inp["w_score"], dtype=F32)

    def w(name):
        return np.asarray(inp[name], dtype=F32)

    def wpair(W):  # [256, 512] = [W | W @ W_sem], bf16
        return np.concatenate([W, W @ W_sem], axis=1).astype(BF16)

    pr["wp_wr"] = wpair(w("W_rel_writes"))
    pr["wp_wn"] = wpair(w("W_rel_written"))
    pr["wp_sp"] = wpair(w("W_self_paper"))
    pr["wp_sa"] = wpair(w("W_self_author"))

    rep = lambda v: np.tile(v.astype(F32), (P, 1))
    pr["bp_p"] = rep(np.concatenate([w("b_self_paper"),
                                     w("b_self_paper") @ W_sem + b_sem]))
    pr["bp_a"] = rep(np.concatenate([w("b_self_author"),
                                     w("b_self_author") @ W_sem + b_sem]))
    pr["bsem"] = rep(b_sem)
    pr["wsc"] = rep(w_score)

    pr["iota"] = np.tile(np.arange(P, dtype=F32), (P, 1)).astype(BF16)
    pr["ident"] = np.eye(P, dtype=F32).astype(BF16)

    # per-core transposed x slices in slot order (self path of the dst shard)
    xta_c, xtp_c = [], []
    xaT = np.zeros((D, NW_ALLOC * P), dtype=BF16)
    xpT = np.zeros((D, NW_ALLOC * P), dtype=BF16)
    xaT[:, :N] = xa.T
    xpT[:, :N] = xp.T
    for c in range(NCORES):
        sp = np.empty((D, NPAD), dtype=BF16)
        sa = np.empty((D, NPAD), dtype=BF16)
        for i in range(NWIN):
            wp_ = int(pr["wr"]["wlist"][i, c])
            wa_ = int(pr["wn"]["wlist"][i, c])
            sp[:, i * P:(i + 1) * P] = xpT[:, wp_ * P:(wp_ + 1) * P]
            sa[:, i * P:(i + 1) * P] = xaT[:, wa_ * P:(wa_ + 1) * P]
        xtp_c.append(sp)
        xta_c.append(sa)
    pr["xtp"], pr["xta"] = xtp_c, xta_c
    return pr


# ---------------------------------------------------------------- program
def build_program(key):
    nwin, BLwr, BHwr, BLwn, BHwn = key
    f32 = mybir.dt.float32
    bf16 = mybir.dt.bfloat16
    i16 = mybir.dt.int16
    AF = mybir.ActivationFunctionType
    OP = mybir.AluOpType

    C_wr = sum(BLwr) + sum(BHwr)
    C_wn = sum(BLwn) + sum(BHwn)
    L16, W16 = _layout16(C_wr, C_wn)
    Lf, Wf = _layoutf()
    Wi = 8 * (C_wr + C_wn)
    npad = nwin * P
    CALLMAX = max(max(a + b for a, b in zip(BLwr, BHwr)),
                  max(a + b for a, b in zip(BLwn, BHwn)))

    coff_wr = np.concatenate([[0], np.cumsum([a + b for a, b in zip(BLwr, BHwr)])])
    coff_wn = np.concatenate([[0], np.cumsum([a + b for a, b in zip(BLwn, BHwn)])])

    nc = bacc.Bacc("TRN2", target_bir_lowering=False, debug=False)

    xb = nc.dram_tensor("xb", [2 * N, D], GDT, kind="ExternalInput")
    pk16 = nc.dram_tensor("pk16", [P, W16], bf16, kind="ExternalInput")
    pki = nc.dram_tensor("pki", [P, Wi], i16, kind="ExternalInput")
    pkf = nc.dram_tensor("pkf", [P, Wf], f32, kind="ExternalInput")
    o = nc.dram_tensor("o", [2 * npad, D], f32, kind="ExternalOutput")

    with tile.TileContext(nc) as tc:
        with tc.tile_pool(name="const", bufs=1) as cpool, \
             tc.tile_pool(name="gbuf", bufs=4) as gpool, \
             tc.tile_pool(name="oh", bufs=4) as ohpool, \
             tc.tile_pool(name="sb", bufs=4) as sbpool, \
             tc.tile_pool(name="mps", bufs=2, space="PSUM") as mpool, \
             tc.tile_pool(name="tps", bufs=1, space="PSUM") as tpool, \
             tc.tile_pool(name="dps", bufs=2, space="PSUM") as dpool:

            def load16(name):
                off, w_ = L16[name]
                t = cpool.tile([P, w_], bf16, tag=f"c_{name}")
                nc.sync.dma_start(t[:], pk16[0:P, off:off + w_])
                return t

            def loadf(name):
                off, w_ = Lf[name]
                t = cpool.tile([P, w_], f32, tag=f"c_{name}")
                nc.sync.dma_start(t[:], pkf[0:P, off:off + w_])
                return t

            t16 = {n: load16(n) for n in L16}
            tf = {n: loadf(n) for n in Lf}
            idx_wr_t = cpool.tile([P, 8 * C_wr], i16, tag="c_idxwr")
            nc.sync.dma_start(idx_wr_t[:], pki[0:P, 0:8 * C_wr])
            idx_wn_t = cpool.tile([P, 8 * C_wn], i16, tag="c_idxwn")
            nc.sync.dma_start(idx_wn_t[:], pki[0:P, 8 * C_wr:Wi])

            rels = [
                dict(BL=BLwr, BH=BHwr, coff=coff_wr, idx=idx_wr_t,
                     colf=t16["colf_wr"], recip=tf["recip_wr"],
                     lo_tab=xb[0:N, :], hi_tab=xb[HALF:N, :],
                     xt=(t16["xtp0"], t16["xtp1"]),
                     wp_rel=(t16["wp_wr0"], t16["wp_wr1"]),
                     wp_self=(t16["wp_sp0"], t16["wp_sp1"]),
                     bp=tf["bp_p"], obase=0),
                dict(BL=BLwn, BH=BHwn, coff=coff_wn, idx=idx_wn_t,
                     colf=t16["colf_wn"], recip=tf["recip_wn"],
                     lo_tab=xb[N:2 * N, :], hi_tab=xb[N + HALF:2 * N, :],
                     xt=(t16["xta0"], t16["xta1"]),
                     wp_rel=(t16["wp_wn0"], t16["wp_wn1"]),
                     wp_self=(t16["wp_sa0"], t16["wp_sa1"]),
                     bp=tf["bp_a"], obase=npad),
            ]

            iota_t = t16["iota"]
            ident_t = t16["ident"]
            bsem_t = tf["bsem"]
            wsc_t = tf["wsc"]

            def emit_slot(i, r):
                BL, BH = r["BL"][i], r["BH"][i]
                call = BL + BH
                ic0 = 8 * int(r["coff"][i])
                co0 = int(r["coff"][i])

                g = gpool.tile([P, CALLMAX, D], GDT, tag="g")
                if BL:
                    nc.gpsimd.dma_gather(
                        g[:, 0:BL, :], r["lo_tab"],
                        r["idx"][:, ic0: ic0 + 8 * BL],
                        BL * P, BL * P, D, single_packet=False)
                if BH:
                    nc.gpsimd.dma_gather(
                        g[:, BL:call, :], r["hi_tab"],
                        r["idx"][:, ic0 + 8 * BL: ic0 + 8 * call],
                        BH * P, BH * P, D, single_packet=False)

                oh = ohpool.tile([P, CALLMAX, P], GDT, tag="oh")
                nc.vector.tensor_tensor(
                    out=oh[:, 0:call, :],
                    in0=r["colf"][:, co0: co0 + call, None].to_broadcast([P, call, P]),
                    in1=iota_t[:, None, :].to_broadcast([P, call, P]),
                    op=OP.is_equal)

                m_ps = mpool.tile([P, D], f32, tag="m")
                pairs = call // 2
                for k in range(pairs):
                    nc.tensor.matmul(out=m_ps[:], lhsT=oh[:, 2 * k:2 * k + 2, :],
                                     rhs=g[:, 2 * k:2 * k + 2, :],
                                     start=(k == 0),
                                     stop=(2 * pairs == call and k == pairs - 1),
                                     perf_mode=mybir.MatmulPerfMode.DoubleRow)
                if call % 2:
                    nc.tensor.matmul(out=m_ps[:], lhsT=oh[:, call - 1, :],
                                     rhs=g[:, call - 1, :],
                                     start=(call == 1), stop=True)

                m_sb = sbpool.tile([P, D], bf16, tag="m_sb")
                nc.scalar.copy(out=m_sb[:], in_=m_ps[:])

                mt = []
                for h2 in range(2):
                    t_ps = tpool.tile([P, P], bf16, tag="t")
                    nc.tensor.transpose(out=t_ps[:], in_=m_sb[:, h2 * P:(h2 + 1) * P],
                                        identity=ident_t[:])
                    mt_sb = sbpool.tile([P, P], bf16, tag=f"mt{h2}")
                    nc.scalar.copy(out=mt_sb[:], in_=t_ps[:])
                    mt.append(mt_sb)

                AS = dpool.tile([P, 512], f32, tag="AS")
                nc.tensor.matmul(out=AS[:], lhsT=mt[0][:], rhs=r["wp_rel"][0][:],
                                 start=True, stop=False)
                nc.tensor.matmul(out=AS[:], lhsT=mt[1][:], rhs=r["wp_rel"][1][:],
                                 start=False, stop=True)

                HS = dpool.tile([P, 512], f32, tag="HS")
                xsl0 = r["xt"][0][:, i * P:(i + 1) * P]
                xsl1 = r["xt"][1][:, i * P:(i + 1) * P]
                nc.tensor.matmul(out=HS[:], lhsT=xsl0, rhs=r["wp_self"][0][:],
                                 start=True, stop=False)
                nc.tensor.matmul(out=HS[:], lhsT=xsl1, rhs=r["wp_self"][1][:],
                                 start=False, stop=True)

                rc = r["recip"][:, i:i + 1]

                # score arg for agg candidate: tanh(recip*sarg + b_sem)
                sargb = sbpool.tile([P, D], f32, tag="sargb")
                nc.vector.scalar_tensor_tensor(
                    out=sargb[:], in0=AS[:, D:2 * D], scalar=rc, in1=bsem_t[:],
                    op0=OP.mult, op1=OP.add)
                t_a = sbpool.tile([P, D], f32, tag="t_a")
                nc.scalar.activation(out=t_a[:], in_=sargb[:], func=AF.Tanh)

                agg_sb = sbpool.tile([P, D], f32, tag="agg_sb")
                nc.scalar.activation(out=agg_sb[:], in_=AS[:, 0:D],
                                     func=AF.Identity, scale=rc)

                HS_sb = sbpool.tile([P, 512], f32, tag="HS_sb")
                nc.vector.tensor_add(out=HS_sb[:], in0=HS[:], in1=r["bp"][:])
                t_h = sbpool.tile([P, D], f32, tag="t_h")
                nc.scalar.activation(out=t_h[:], in_=HS_sb[:, D:2 * D], func=AF.Tanh)

                scr_a = sbpool.tile([P, D], f32, tag="scr_a")
                s_a = sbpool.tile([P, 1], f32, tag="s_a")
                nc.vector.affine_mul_reduce(out=scr_a[:], accum_out=s_a[:],
                                            in0=t_a[:], in1=wsc_t[:],
                                            scale=1.0, bias=0.0)
                scr_h = sbpool.tile([P, D], f32, tag="scr_h")
                s_h = sbpool.tile([P, 1], f32, tag="s_h")
                nc.vector.affine_mul_reduce(out=scr_h[:], accum_out=s_h[:],
                                            in0=t_h[:], in1=wsc_t[:],
                                            scale=1.0, bias=0.0)

                dsc = sbpool.tile([P, 1], f32, tag="dsc")
                nc.vector.tensor_sub(out=dsc[:], in0=s_h[:], in1=s_a[:])
                a0 = sbpool.tile([P, 1], f32, tag="a0")
                nc.scalar.activation(out=a0[:], in_=dsc[:], func=AF.Sigmoid)

                diff = sbpool.tile([P, D], f32, tag="diff")
                nc.vector.tensor_sub(out=diff[:], in0=HS_sb[:, 0:D], in1=agg_sb[:])
                outt = sbpool.tile([P, D], f32, tag="outt")
                nc.vector.scalar_tensor_tensor(
                    out=outt[:], in0=diff[:], scalar=a0[:, 0:1], in1=agg_sb[:],
                    op0=OP.mult, op1=OP.add)
                nc.sync.dma_start(o[r["obase"] + i * P: r["obase"] + (i + 1) * P, :],
                                  outt[:])

            for i in range(nwin):
                for r in rels:
                    emit_slot(i, r)

    nc.compile()
    return nc


# ---------------------------------------------------------------- driver
_PROG_CACHE = {}


def _get_program(key):
    if key not in _PROG_CACHE:
        _PROG_CACHE[key] = build_program(key)
    return _PROG_CACHE[key]


def _prog_key(pr):
    return (NWIN, pr["wr"]["BL"], pr["wr"]["BH"], pr["wn"]["BL"], pr["wn"]["BH"])


def _make_in_maps(pr):
    C_wr, C_wn = pr["wr"]["C"], pr["wn"]["C"]
    L16, W16 = _layout16(C_wr, C_wn)
    Lf, Wf = _layoutf()
    in_maps = []
    for c in range(NCORES):
        p16 = np.zeros((P, W16), dtype=BF16)

        def put16(name, arr):
            off, w_ = L16[name]
            p16[:, off:off + w_] = arr

        put16("iota", pr["iota"])
        put16("ident", pr["ident"])
        put16("wp_wr0", pr["wp_wr"][0:P])
        put16("wp_wr1", pr["wp_wr"][P:D])
        put16("wp_wn0", pr["wp_wn"][0:P])
        put16("wp_wn1", pr["wp_wn"][P:D])
        put16("wp_sp0", pr["wp_sp"][0:P])
        put16("wp_sp1", pr["wp_sp"][P:D])
        put16("wp_sa0", pr["wp_sa"][0:P])
        put16("wp_sa1", pr["wp_sa"][P:D])
        put16("colf_wr", pr["wr"]["colf"][c])
        put16("colf_wn", pr["wn"]["colf"][c])
        put16("xtp0", pr["xtp"][c][0:P])
        put16("xtp1", pr["xtp"][c][P:D])
        put16("xta0", pr["xta"][c][0:P])
        put16("xta1", pr["xta"][c][P:D])

        pf = np.zeros((P, Wf), dtype=F32)

        def putf(name, arr):
            off, w_ = Lf[name]
            pf[:, off:off + w_] = arr

        putf("bp_p", pr["bp_p"])
        putf("bp_a", pr["bp_a"])
        putf("bsem", pr["bsem"])
        putf("wsc", pr["wsc"])
        putf("recip_wr", pr["wr"]["recip"][c])
        putf("recip_wn", pr["wn"]["recip"][c])

        pi = np.concatenate([pr["wr"]["idx"][c], pr["wn"]["idx"][c]], axis=1)
        in_maps.append(dict(xb=pr["xb"], pk16=p16, pki=np.ascontiguousarray(pi),
                            pkf=pf))
    return in_maps


def run(trace=False, tmpdir=None, **inputs):
    pr = _host_prep(inputs)
    nc = _get_program(_prog_key(pr))
    in_maps = _make_in_maps(pr)
    res = run_bass_kernel_spmd(nc, in_maps, list(range(NCORES)),
                               trace=trace, tmpdir=tmpdir)
    oa = np.empty((N, D), dtype=F32)
    op = np.empty((N, D), dtype=F32)
    for c in range(NCORES):
        ores = res.results[c]["o"]
        for i in range(NWIN):
            wp_ = int(pr["wr"]["wlist"][i, c])
            if wp_ < NW_TOTAL:
                r0, r1 = wp_ * P, min(N, (wp_ + 1) * P)
                op[r0:r1] = ores[i * P: i * P + (r1 - r0)]
            wa_ = int(pr["wn"]["wlist"][i, c])
            if wa_ < NW_TOTAL:
                r0, r1 = wa_ * P, min(N, (wa_ + 1) * P)
                oa[r0:r1] = ores[NPAD + i * P: NPAD + i * P + (r1 - r0)]
    return (oa, op), res


def kernel(**inputs):
    (oa, op), _ = run(trace=False, **inputs)
    return (oa, op)
